# revision 1
# baseline (speedup 1.0000x reference)
"""Trainium2 Bass kernel for one burst-mode CIF neuron step.

Reference math (closed form of the two burst while-loops):
    m      = mem + x
    k_pos  = max(ceil((m - th)/th), 0)            # positive burst count
    m1     = m - k_pos*th
    scu    = round((spike_count + k_pos*th)/th)
    j_mem  = max(ceil((-m1 - th)/th), 0)
    k_neg  = min(j_mem, max(scu, 0))
    spike  = (k_pos - k_neg)*th

On-device reformulation: the two loops are mutually exclusive (if k_pos > 0
then m1 in (0, th], so j_mem = 0).  With q = m/th:
    k_pos = relu(ceil(q) - 1)
    k_neg = min(relu(-floor(q) - 1), spike_count/th)
    spike = (k_pos - k_neg)*th
(spike_count/th is a non-negative near-integer; using it unrounded inside
the min only perturbs the result by ~1 ulp when it wins the min.)

ceil/floor come from the fp32 round-to-nearest magic constant C = 1.5*2^23:
for |v| < 2^22, rint(v) == (v + C) - C.  Let ts2 = (q + 0.5) + C
= C + ceil(q) (exact except q an exact odd integer, measure-zero on this
data and equally boundary-sensitive in the reference):
    k_pos = relu(ts2 - (C+1))
    j_mem = relu(-floor(q) - 1) = relu(-(ceil(q)-1) - 1) = relu(-ts2 + C)

Sharding: pure elementwise -> flatten [B,T,H] to [B*T, H] rows and shard
rows across the 8 cores data-parallel; threshold [H] replicated.

Dtypes: x/mem stay f32 (the ceil/floor boundaries need full precision).
spike_count is cast to bf16 on the host (exact through the min: counts
<= 3, bf16 error ~0.4% << the 0.5 rounding margin) and the spike output
is stored bf16 and upcast on the host.  Measured end-to-end L2 relative
error vs the jax reference: 1.7e-3 (bf16 output rounding; the f32
variant measures 1.3e-4, limited only by a handful of 67M elements
sitting exactly on ceil boundaries where the reference itself is
1-ulp-sensitive).

The memory layout packs, per row and per column-half, [x_half(f32) |
mem_half(f32) | sc_half(bf16 bits)] into one f32 input tensor so each
[128, 2048] work tile arrives in ONE contiguous 20KB/partition-row DMA.
This matters because the hardware allows one semaphore wait per
instruction (Bacc's generate_event_semaphores splits the rest, at a
cost), so the dataflow is arranged so nearly every op has at most one
unobserved cross-engine dependency, with tiny observer copies
pre-observing the others.

Engine split per [128, 2048] half-tile (measured on HW: DMA ~300-340us,
DVE ~325us, Pool ~265us, ACT ~254us busy per core; e2e ~482us vs a
~375us pure-HBM roofline):
    DVE : m = x+mem, kn = min(jm,s), spike = d*TH
    ACT : ta = q+0.5, ts2 = ta+C, kp = Relu(ts2-(C+1)), jm = Relu(-ts2+C)
    Pool: q = m*R (half 0; half 1 on DVE to balance), s = sc*R
    PE  : d = kp - kn via [I|-I] matmuls into PSUM (exact: small ints)
"""

import numpy as np

B, T, H = 4, 4096, 4096
N_CORES = 8
ROWS_PER_CORE = (B * T) // N_CORES  # 2048
P = 128
HALF = 2048
MAGIC = 12582912.0  # 1.5 * 2^23

_NC_CACHE: dict = {}


def pack_inputs(x, mem, sc_bf16):
    """Pack per row, per column-half: [x_half | mem_half | sc_half(bf16)].

    Gives the kernel exactly one contiguous 20 KB/partition-row DMA per
    [128, 2048] tile (the HW allows one semaphore wait per instruction,
    so a multi-input op may depend on at most one fresh DMA).
    """
    rows = x.shape[0]
    grp = 2 * HALF + HALF // 2
    xm = np.empty((rows, 2 * grp), dtype=np.float32)
    for hf in range(2):
        c0, o0 = hf * HALF, hf * grp
        xm[:, o0 : o0 + HALF] = x[:, c0 : c0 + HALF]
        xm[:, o0 + HALF : o0 + 2 * HALF] = mem[:, c0 : c0 + HALF]
        xm[:, o0 + 2 * HALF : o0 + grp] = np.ascontiguousarray(
            sc_bf16[:, c0 : c0 + HALF]
        ).view(np.float32)
    return xm


def build_nc(rows: int = ROWS_PER_CORE):
    """Build the per-core Bass program (identical on all cores)."""
    from contextlib import ExitStack

    import concourse.bacc as bacc
    import concourse.bass as bass
    import concourse.mybir as mybir
    from bass_rust import add_dep_helper
    from concourse.tile import TileContext

    f32 = mybir.dt.float32
    Alu = mybir.AluOpType
    Act = mybir.ActivationFunctionType

    assert rows % P == 0
    n_blocks = rows // P

    bf16 = mybir.dt.bfloat16
    # packed row: per column-half hf: [x_half (HALF f32) | mem_half (HALF
    # f32) | spike_count_half (HALF bf16 = HALF/2 f32 words)]
    GRP = 2 * HALF + HALF // 2  # 5120 f32 words per halfgroup
    nc = bacc.Bacc("TRN2", target_bir_lowering=False, debug=False)
    xm_d = nc.dram_tensor("xm", [rows, 2 * GRP], f32, kind="ExternalInput").ap()
    t_d = nc.dram_tensor("threshold", [H], f32, kind="ExternalInput").ap()
    e_d = nc.dram_tensor("eye", [P, 2 * P], bf16, kind="ExternalInput").ap()
    o_d = nc.dram_tensor("spike", [rows, H], bf16, kind="ExternalOutput").ap()
    r_d = nc.dram_tensor("recip_scratch", [H], f32, kind="Internal").ap()

    with TileContext(nc) as tc, ExitStack() as ctx:
        consts = ctx.enter_context(tc.tile_pool(name="consts", bufs=1))
        io = ctx.enter_context(tc.tile_pool(name="io", bufs=2))
        io3 = ctx.enter_context(tc.tile_pool(name="io3", bufs=3))
        work = ctx.enter_context(tc.tile_pool(name="work", bufs=1))
        work2 = ctx.enter_context(tc.tile_pool(name="work2", bufs=2))
        work3 = ctx.enter_context(tc.tile_pool(name="work3", bufs=3))
        once = ctx.enter_context(tc.tile_pool(name="once", bufs=1))

        # ---- one-time threshold setup ----
        # th broadcast to all 128 partitions via a step-0 partition DMA.
        TH = consts.tile([P, H], f32, tag="TH")
        th_bcast = bass.AP(
            tensor=t_d.tensor, offset=t_d.offset, ap=[[0, P]] + list(t_d.ap)
        )
        nc.gpsimd.dma_start(out=TH[:], in_=th_bcast)

        # reciprocal computed on a [128, H/128] relayout (H distinct values,
        # not 128*H), bounced through DRAM, then broadcast like th.
        th_pn = consts.tile([P, H // P], f32, tag="th_pn")
        nc.sync.dma_start(out=th_pn[:], in_=t_d.rearrange("(n p) -> p n", p=P))
        r_pn = consts.tile([P, H // P], f32, tag="r_pn")
        nc.vector.reciprocal(r_pn[:], th_pn[:])
        nc.sync.dma_start(out=r_d.rearrange("(n p) -> p n", p=P), in_=r_pn[:])
        R = consts.tile([P, H], f32, tag="R")
        r_bcast = bass.AP(
            tensor=r_d.tensor, offset=r_d.offset, ap=[[0, P]] + list(r_d.ap)
        )
        nc.gpsimd.dma_start(out=R[:], in_=r_bcast)

        # per-partition bias vectors for the ACT ops
        bias_kp = consts.tile([P, 1], f32, tag="bias_kp")
        nc.vector.memset(bias_kp[:], -(MAGIC + 1.0))
        bias_jm = consts.tile([P, 1], f32, tag="bias_jm")
        nc.vector.memset(bias_jm[:], MAGIC)
        bias_half = consts.tile([P, 1], f32, tag="bias_half")
        nc.vector.memset(bias_half[:], 0.5)
        bias_C = consts.tile([P, 1], f32, tag="bias_C")
        nc.vector.memset(bias_C[:], MAGIC)

        # Pool pre-touches R so its later ops never wait on R's DMA.
        pool_dummy = consts.tile([P, 1], f32, tag="pool_dummy")
        nc.gpsimd.tensor_copy(pool_dummy[:], R[:, 0:1])

        dve_dummy = consts.tile([P, 1], f32, tag="dve_dummy")

        # [I | -I] for the PE-side d = kp - kn (exact: kp/kn are small ints)
        eye = consts.tile([P, 2 * P], bf16, tag="eye")
        nc.sync.dma_start(out=eye[:], in_=e_d)
        psum = ctx.enter_context(tc.tile_pool(name="psum", bufs=2, space="PSUM"))
        NMM = 512  # matmul free-dim cap (one PSUM bank)

        # ---- main loop: n_blocks row-blocks x 2 column halves ----
        xm_t = xm_d.rearrange("(nb p) (hh w) -> nb hh p w", p=P, hh=2, w=GRP)
        o_t = o_d.rearrange("(nb p) h -> nb p h", p=P)

        for b in range(n_blocks):
            tout = io.tile([P, H], bf16, tag="out")

            for hf in range(2):
                sl = bass.ts(hf, HALF)
                txm = io3.tile([P, GRP], f32, tag="xm")
                nc.sync.dma_start(out=txm[:], in_=xm_t[b, hf])
                xpart = txm[:, 0:HALF]
                mpart = txm[:, HALF : 2 * HALF]
                scb = txm[:, 2 * HALF : GRP].bitcast(bf16)

                # m = x + mem   (DVE; only cross dep = the xm DMA)
                tm = work.tile([P, HALF], f32, tag="tm")
                nc.vector.tensor_tensor(tm[:], xpart, mpart, Alu.add)
                # q = m * (1/th), in place; alternate Pool/DVE to balance
                # (Pool work also steals a shared SBUF port from DVE).
                if hf == 0:
                    nc.gpsimd.tensor_tensor(tm[:], tm[:], R[:, sl], Alu.mult)
                else:
                    nc.vector.tensor_tensor(tm[:], tm[:], R[:, sl], Alu.mult)
                # rounding chain on ACT (own SBUF port, plenty of headroom):
                # ta = q + 0.5 ; ts2 = ta + C = C + ceil(q)
                ta = work2.tile([P, HALF], f32, tag="tab")
                nc.scalar.activation(ta[:], tm[:], Act.Identity, bias=bias_half[:])
                tb = work2.tile([P, HALF], f32, tag="tab")
                nc.scalar.activation(tb[:], ta[:], Act.Identity, bias=bias_C[:])
                # k_pos = relu(ts2 - (C+1)), then j_mem = relu(-ts2 + C); kp
                # first so the min's wait on jm's tick also covers kp.
                # kp/jm/kn/s are small integers (+-0.4% on s) -> bf16-exact;
                # bf16 gets the min into the DVE 2x_1P mode and halves the
                # PE operand bytes.
                tkq = work3.tile([P, HALF], bf16, tag="tkq")
                nc.scalar.activation(tkq[:], tb[:], Act.Relu, bias=bias_kp[:])
                tj = work3.tile([P, HALF], bf16, tag="tj")
                nc.scalar.activation(
                    tj[:], tb[:], Act.Relu, bias=bias_jm[:], scale=-1.0
                )
                # s = spike_count / th   (Pool; bf16 in/out)
                tsn = work2.tile([P, HALF], bf16, tag="tsn")
                nc.gpsimd.tensor_tensor(tsn[:], scb, R[:, sl], Alu.mult)
                # DVE pre-observes Pool's s tick, then the min needs only
                # the ACT wait.
                i_obs = nc.vector.tensor_copy(dve_dummy[:], tsn[:, 0:1])
                # kn = min(j_mem, s)
                i_min = nc.vector.tensor_tensor(tj[:], tj[:], tsn[:], Alu.min)
                add_dep_helper(i_min.ins, i_obs.ins, sync=False, reason="obs<min")
                # d = k_pos - kn on the (otherwise idle) TensorEngine:
                # psum = I.T @ kp + (-I).T @ kn, exact for small integers.
                td = psum.tile([P, HALF], f32, tag="td")
                for c in range(HALF // NMM):
                    cs = bass.ts(c, NMM)
                    nc.tensor.matmul(
                        td[:, cs], eye[:, 0:P], tkq[:, cs], start=True, stop=False
                    )
                    nc.tensor.matmul(
                        td[:, cs], eye[:, P : 2 * P], tj[:, cs],
                        start=False, stop=True,
                    )
                # spike = d * th
                nc.vector.tensor_tensor(tout[:, sl], td[:], TH[:, sl], Alu.mult)

            nc.sync.dma_start(out=o_t[b], in_=tout[:])

    return nc


def kernel(**inputs: np.ndarray) -> np.ndarray:
    import ml_dtypes

    from concourse.bass_utils import run_bass_kernel_spmd

    x = np.ascontiguousarray(inputs["x"], dtype=np.float32).reshape(B * T, H)
    mem = np.ascontiguousarray(inputs["mem"], dtype=np.float32).reshape(B * T, H)
    sc = (
        np.ascontiguousarray(inputs["spike_count"], dtype=np.float32)
        .reshape(B * T, H)
        .astype(ml_dtypes.bfloat16)
    )
    th = np.ascontiguousarray(inputs["threshold"], dtype=np.float32)
    xm = pack_inputs(x, mem, sc)
    eye = np.concatenate(
        [np.eye(P, dtype=np.float32), -np.eye(P, dtype=np.float32)], axis=1
    ).astype(ml_dtypes.bfloat16)

    if "nc" not in _NC_CACHE:
        nc = build_nc()
        nc.finalize()
        _NC_CACHE["nc"] = nc
    nc = _NC_CACHE["nc"]

    r = ROWS_PER_CORE
    in_maps = [
        {
            "xm": xm[c * r : (c + 1) * r],
            "threshold": th,
            "eye": eye,
        }
        for c in range(N_CORES)
    ]
    res = run_bass_kernel_spmd(nc, in_maps, core_ids=list(range(N_CORES)))
    out = np.concatenate(
        [res.results[c]["spike"].astype(np.float32) for c in range(N_CORES)], axis=0
    )
    return out.reshape(B, T, H)

